# revision 2
# baseline (speedup 1.0000x reference)
"""Grouped multi-query attention on 8 trn2 NeuronCores — head-sharded.

Sharding: 8 cores = batch (2) x kv-head-pairs (4). Core c handles batch
g = c//4 and kv heads {2hp, 2hp+1} (hp = c%4) with their 8 q heads, over
the FULL sequence (n = 2048). The output projection is row-parallel: each
core computes a partial [n, d_model] product over its 512 o-features; the
host sums the 4 partials per batch (and adds b_o). No device collectives.

All matmul inputs are staged bf16 (halves DMA + SBUF). Layouts are
d-major so every contraction sits on the partition axis. Score matmuls
for the two kv heads run CONCURRENTLY in the PE array via row-group
tiling (d_k=64 contraction -> rows 0-63 / 64-127, tile_position derived
from base partitions). Softmax denominators come from a ones-column
appended to each head's v tile; exp runs on the scalar engine (the
critical resource, ~1 elem/lane/cycle) at 1024-free into bf16; the
1/denom row-broadcast runs on the otherwise-idle GPSIMD engine.

The q projection runs FIRST (streamed against the Qt DMA with all 8 PSUM
banks as dc-accumulators) so attention's exp stream starts as early as
possible; k/v projections and attn@v join in as their inputs land.
"""
import sys
sys.path.insert(0, '/opt/trn_rl_repo')

import numpy as np

D = 2048          # d_model
N = 2048          # sequence length
B = 2             # batch
KVH = 8           # kv heads total
QH = 32           # q heads total
DK = 64           # head dim
HPC = 8           # q heads per core
FPC = HPC * DK    # 512 o-features per core
USE_GPSIMD_BCAST = True
_NC_CACHE = {}


def _build_nc(reps=1, phases="all"):
    import concourse.bacc as bacc
    import concourse.mybir as mybir
    from concourse import tile

    F32 = mybir.dt.float32
    F32R = mybir.dt.float32r
    BF16 = mybir.dt.bfloat16

    nc = bacc.Bacc("TRN2", target_bir_lowering=False, debug=False)

    qt = nc.dram_tensor("qt", [D, N], BF16, kind="ExternalInput").ap()
    kt = nc.dram_tensor("kt", [D, N], BF16, kind="ExternalInput").ap()
    vt = nc.dram_tensor("vt", [D, N], BF16, kind="ExternalInput").ap()
    wqt = nc.dram_tensor("wqt", [D, FPC], BF16, kind="ExternalInput").ap()
    wkt = nc.dram_tensor("wkt", [D, 128], BF16, kind="ExternalInput").ap()
    wvt = nc.dram_tensor("wvt", [D, 128], BF16, kind="ExternalInput").ap()
    wot = nc.dram_tensor("wot", [FPC, D], BF16, kind="ExternalInput").ap()
    eye = nc.dram_tensor("eye", [128, 128], F32, kind="ExternalInput").ap()
    outt = nc.dram_tensor("outt", [N, D], BF16, kind="ExternalOutput").ap()

    def body(tc):
        with (
            tc.tile_pool(name="persist", bufs=1) as persist,
            tc.tile_pool(name="wpool", bufs=1) as wpool,
            tc.tile_pool(name="small", bufs=3) as small,
        ):
            # persistent SBUF
            qt_all = persist.tile([128, 4 * N], BF16, tag="qt_all")
            kt_all = persist.tile([128, N], BF16, tag="kt_all")
            v_nat = persist.tile([128, 16 * 130], F32R, tag="v_nat")
            ot_all = persist.tile([128, 4 * N], BF16, tag="ot_all")
            eye_sb = persist.tile([128, 128], F32, tag="eye")
            ones_pb = persist.tile([1, 64], BF16, tag="ones_pb")
            ones_f = persist.tile([128, 32], F32, tag="ones_f")

            nc.vector.memset(ones_pb[:], 1.0)
            nc.vector.memset(ones_f[:], 1.0)
            nc.sync.dma_start(eye_sb[:], eye[:])
            # ones column after each head's 64 v-columns (rounding copy
            # because memset cannot write f32r directly)
            ones_view = v_nat[:].rearrange("p (a c) -> p a c", a=32, c=65)
            nc.vector.tensor_copy(ones_view[:, :, 64:65], ones_f[:])

            # weights (wo emitted last on the DMA queue — needed latest)
            wq_sb = wpool.tile([128, 16 * FPC], BF16, tag="wq_sb")
            wk_sb = wpool.tile([128, 16 * 128], BF16, tag="wk_sb")
            wv_sb = wpool.tile([128, 16 * 128], BF16, tag="wv_sb")
            wo_sb = wpool.tile([128, 4 * D], BF16, tag="wo_sb")
            for dc in range(16):
                nc.sync.dma_start(wq_sb[:, dc * FPC:(dc + 1) * FPC],
                                  wqt[dc * 128:(dc + 1) * 128, :])
            nc.sync.dma_start(wk_sb[:].rearrange("p (a c) -> p a c", a=16),
                              wkt[:].rearrange("(a p) c -> p a c", p=128))
            nc.sync.dma_start(wv_sb[:].rearrange("p (a c) -> p a c", a=16),
                              wvt[:].rearrange("(a p) c -> p a c", p=128))

            # ---------------- q projection (first: attention critical path)
            with tc.tile_pool(name="qtin", bufs=1) as qtin:
                qt_in = qtin.tile([128, 16 * N], BF16, tag="qt_in")
                for dc in range(16):
                    nc.sync.dma_start(qt_in[:, dc * N:(dc + 1) * N],
                                      qt[dc * 128:(dc + 1) * 128, :])
                with tc.tile_pool(name="qpsum", bufs=1, space="PSUM") as qpsum:
                    for half in range(2):
                        ps = [qpsum.tile([128, 512], F32, tag=f"qp{i}",
                                         name=f"qp{i}")
                              for i in range(8)]
                        for dc in range(16):
                            for fc in range(4):
                                for nq in range(2):
                                    nqa = half * 2 + nq
                                    nc.tensor.matmul(
                                        ps[fc * 2 + nq][:],
                                        wq_sb[:, dc * FPC + fc * 128:
                                              dc * FPC + (fc + 1) * 128],
                                        qt_in[:, dc * N + nqa * 512:
                                              dc * N + (nqa + 1) * 512],
                                        start=(dc == 0), stop=(dc == 15))
                        for fc in range(4):
                            for nq in range(2):
                                nqa = half * 2 + nq
                                nc.vector.tensor_copy(
                                    qt_all[:, fc * N + nqa * 512:
                                           fc * N + (nqa + 1) * 512],
                                    ps[fc * 2 + nq][:])

                # ------------ k and v projections (streamed)
                with (
                    tc.tile_pool(name="kvpsum", bufs=2, space="PSUM") as kvpsum,
                    tc.tile_pool(name="kvstream", bufs=3) as kvs,
                ):
                    vp = kvpsum.tile([128, N], F32, tag="kvp", name="vp")
                    for dc in range(16):
                        vc = kvs.tile([128, N], BF16, tag="kv_in")
                        nc.sync.dma_start(vc[:], vt[dc * 128:(dc + 1) * 128, :])
                        for nq in range(4):
                            nc.tensor.matmul(
                                vp[:, nq * 512:(nq + 1) * 512],
                                wv_sb[:, dc * 128:(dc + 1) * 128],
                                vc[:, nq * 512:(nq + 1) * 512],
                                start=(dc == 0), stop=(dc == 15))
                    vtmp = qtin.tile([128, N], F32, tag="vtmp")
                    nc.vector.tensor_copy(vtmp[:], vp[:])

                    kp = kvpsum.tile([128, N], F32, tag="kvp", name="kp")
                    for dc in range(16):
                        kc = kvs.tile([128, N], BF16, tag="kv_in")
                        nc.sync.dma_start(kc[:], kt[dc * 128:(dc + 1) * 128, :])
                        for nq in range(4):
                            nc.tensor.matmul(
                                kp[:, nq * 512:(nq + 1) * 512],
                                wk_sb[:, dc * 128:(dc + 1) * 128],
                                kc[:, nq * 512:(nq + 1) * 512],
                                start=(dc == 0), stop=(dc == 15))
                    nc.vector.tensor_copy(kt_all[:], kp[:])

                with tc.tile_pool(name="trpsum", bufs=2, space="PSUM") as trps:
                    for mc in range(16):
                        trp = trps.tile([128, 128], F32, tag="trp")
                        nc.tensor.transpose(
                            trp[:], vtmp[:, mc * 128:(mc + 1) * 128], eye_sb[:])
                        base = mc * 130
                        nc.vector.tensor_copy(
                            v_nat[:, base:base + 64], trp[:, 0:64])
                        nc.vector.tensor_copy(
                            v_nat[:, base + 65:base + 129], trp[:, 64:128])

            for fc in range(4):
                nc.sync.dma_start(wo_sb[:, fc * D:(fc + 1) * D],
                                  wot[fc * 128:(fc + 1) * 128, :])

            if phases == "proj":
                return

            # ---------------- attention ----------------
            with (
                tc.tile_pool(name="scp", bufs=2, space="PSUM") as scp,
                tc.tile_pool(name="pop", bufs=2, space="PSUM") as pop,
                tc.tile_pool(name="expp", bufs=6) as expp,
            ):
                for p in range(4):
                    for nh in range(2):
                        n0 = nh * 1024
                        po_a = pop.tile([65, 1024], F32, tag="po", name="poA")
                        po_b = pop.tile([65, 1024], F32, tag="po", name="poB")
                        exp_tiles = []
                        for mc in range(17):
                            if mc < 16:
                                # scores for chunk mc: A (rows 0-63) and B
                                # (rows 64-127) run concurrently via PE
                                # row-group tiling
                                sc_a = scp.tile([128, 1024], F32, tag="sc",
                                                name="scA")
                                sc_b = scp.tile([128, 1024], F32, tag="sc",
                                                name="scB")
                                for nq in range(2):
                                    nc.tensor.matmul(
                                        sc_a[:, nq * 512:(nq + 1) * 512],
                                        kt_all[0:64, mc * 128:(mc + 1) * 128],
                                        qt_all[0:64, p * N + n0 + nq * 512:
                                               p * N + n0 + (nq + 1) * 512],
                                        start=True, stop=True)
                                    nc.tensor.matmul(
                                        sc_b[:, nq * 512:(nq + 1) * 512],
                                        kt_all[64:128, mc * 128:(mc + 1) * 128],
                                        qt_all[64:128, p * N + n0 + nq * 512:
                                               p * N + n0 + (nq + 1) * 512],
                                        start=True, stop=True)
                                ex_a = expp.tile([128, 1024], F32R, tag="exp",
                                                 name="exA")
                                ex_b = expp.tile([128, 1024], F32R, tag="exp",
                                                 name="exB")
                                nc.scalar.activation(
                                    ex_a[:], sc_a[:],
                                    mybir.ActivationFunctionType.Exp)
                                nc.scalar.activation(
                                    ex_b[:], sc_b[:],
                                    mybir.ActivationFunctionType.Exp)
                                exp_tiles.append((ex_a, ex_b))
                            if mc > 0:
                                # attn@v for chunk mc-1 (emitted after the
                                # next scores so PE isn't head-of-line
                                # blocked behind the exp dependency)
                                mm = mc - 1
                                ex_a, ex_b = exp_tiles[mm]
                                for nq in range(2):
                                    nc.tensor.matmul(
                                        po_a[:, nq * 512:(nq + 1) * 512],
                                        v_nat[:, mm * 130:mm * 130 + 65],
                                        ex_a[:, nq * 512:(nq + 1) * 512],
                                        start=(mm == 0), stop=(mm == 15))
                                for nq in range(2):
                                    nc.tensor.matmul(
                                        po_b[:, nq * 512:(nq + 1) * 512],
                                        v_nat[:, mm * 130 + 65:mm * 130 + 130],
                                        ex_b[:, nq * 512:(nq + 1) * 512],
                                        start=(mm == 0), stop=(mm == 15))

                        # normalize: ot = po[0:64] * (1/denom), denom in row 64
                        for hb, po in ((0, po_a), (1, po_b)):
                            rcp = small.tile([1, 1024], F32, tag="rcp")
                            nc.vector.reciprocal(rcp[:], po[64:65, :])
                            rcr = small.tile([1, 1024], BF16, tag="rcr")
                            nc.vector.tensor_copy(rcr[:], rcp[:])
                            bc = small.tile([64, 1024], BF16, tag="bc")
                            if USE_GPSIMD_BCAST:
                                nc.gpsimd.partition_broadcast(bc[:], rcr[:])
                            else:
                                pb = scp.tile([64, 1024], F32, tag="sc",
                                              name="pb")
                                for nq in range(2):
                                    nc.tensor.matmul(
                                        pb[:, nq * 512:(nq + 1) * 512],
                                        ones_pb[:],
                                        rcr[:, nq * 512:(nq + 1) * 512],
                                        start=True, stop=True)
                                nc.vector.tensor_copy(bc[:], pb[:])
                            if hb == 0:
                                nc.vector.tensor_mul(
                                    ot_all[0:64, p * N + n0:p * N + n0 + 1024],
                                    po[0:64, :], bc[:])
                            else:
                                tmp = small.tile([64, 1024], BF16, tag="tmp")
                                nc.vector.tensor_mul(tmp[:], po[0:64, :], bc[:])
                                nc.sync.dma_start(
                                    ot_all[64:128,
                                           p * N + n0:p * N + n0 + 1024],
                                    tmp[:])

            if phases == "noout":
                return

            # ---------------- output projection (row-parallel partial)
            # out_partial[n, j] = sum_fc ot[fc-chunk, n].T @ wo[fc-chunk, j]
            with (
                tc.tile_pool(name="outp", bufs=4, space="PSUM") as outp,
                tc.tile_pool(name="outsb", bufs=4) as outsb,
            ):
                for ncnk in range(16):
                    for jq in range(4):
                        ps = outp.tile([128, 512], F32, tag="op")
                        for fc in range(4):
                            nc.tensor.matmul(
                                ps[:],
                                ot_all[:, fc * N + ncnk * 128:
                                       fc * N + (ncnk + 1) * 128],
                                wo_sb[:, fc * D + jq * 512:
                                      fc * D + (jq + 1) * 512],
                                start=(fc == 0), stop=(fc == 3))
                        ostage = outsb.tile([128, 512], BF16, tag="ostage")
                        if jq % 2 == 0:
                            nc.vector.tensor_copy(ostage[:], ps[:])
                        else:
                            nc.scalar.copy(ostage[:], ps[:])
                        nc.sync.dma_start(
                            outt[ncnk * 128:(ncnk + 1) * 128,
                                 jq * 512:(jq + 1) * 512],
                            ostage[:])

    with tile.TileContext(nc) as tc:
        if reps == 1:
            body(tc)
        else:
            with tc.For_i(0, reps):
                body(tc)
    nc.compile()
    return nc


def get_nc(reps=1, phases="all"):
    key = (reps, phases)
    if key not in _NC_CACHE:
        _NC_CACHE[key] = _build_nc(reps, phases)
    return _NC_CACHE[key]


def _bf16():
    import concourse.mybir as mybir
    return mybir.dt.np(mybir.dt.bfloat16)


def make_in_maps(Q, K, V, w_q, w_k, w_v, w_o):
    bf16 = _bf16()
    scale = 1.0 / np.sqrt(DK)
    # local head order per core: [h0, h4 | h1, h5 | h2, h6 | h3, h7]
    perm = []
    for p in range(4):
        perm += list(range(p * 64, (p + 1) * 64))
        perm += list(range((4 + p) * 64, (5 + p) * 64))
    perm = np.array(perm, dtype=np.int64)
    eye = np.eye(128, dtype=np.float32)

    Qts = [np.ascontiguousarray(np.asarray(Q[g]).T).astype(bf16)
           for g in range(B)]
    Kts = [np.ascontiguousarray(np.asarray(K[g]).T).astype(bf16)
           for g in range(B)]
    Vts = [np.ascontiguousarray(np.asarray(V[g]).T).astype(bf16)
           for g in range(B)]

    in_maps = []
    for c in range(8):
        g, hp = c // 4, c % 4
        rsel = slice(hp * FPC, (hp + 1) * FPC)
        ksel = slice(hp * 128, (hp + 1) * 128)
        wq_l = np.asarray(w_q)[rsel][perm] * scale           # [512, 2048]
        wqt_c = np.ascontiguousarray(wq_l.T).astype(bf16)    # [2048, 512]
        wkt_c = np.ascontiguousarray(np.asarray(w_k)[ksel].T).astype(bf16)
        wvt_c = np.ascontiguousarray(np.asarray(w_v)[ksel].T).astype(bf16)
        wo_l = np.asarray(w_o)[:, rsel][:, perm]             # [2048, 512]
        wot_c = np.ascontiguousarray(wo_l.T).astype(bf16)    # [512, 2048]
        in_maps.append({
            "qt": Qts[g], "kt": Kts[g], "vt": Vts[g],
            "wqt": wqt_c, "wkt": wkt_c, "wvt": wvt_c, "wot": wot_c,
            "eye": eye,
        })
    return in_maps


def assemble_output(res_maps, b_o):
    out = np.zeros((B, N, D), dtype=np.float32)
    for c in range(8):
        g = c // 4
        out[g] += np.asarray(res_maps[c]["outt"], dtype=np.float32)
    out += np.asarray(b_o, dtype=np.float32)[None, None, :]
    return out


def kernel(Q, K, V, w_q, w_k, w_v, w_o, b_o):
    from concourse.bass_utils import run_bass_kernel_spmd
    nc = get_nc()
    in_maps = make_in_maps(Q, K, V, w_q, w_k, w_v, w_o)
    res = run_bass_kernel_spmd(nc, in_maps, core_ids=list(range(8)))
    return assemble_output(res.results, b_o)
